# revision 11
# baseline (speedup 1.0000x reference)
"""TT-linear as dense bf16 GEMM, tuned for PE utilization.

Per core: out[512, 4096] = x[512, 4096] @ W[4096, 4096] + bias.
W is materialized on host from the TT cores (TT contraction has the same
FLOP count as the dense GEMM but runs at lower PE efficiency, so dense wins).

Schedule (per core):
  - xT resident in SBUF as 32 tiles [128, 512] bf16 (4 MB), loaded on the
    gpsimd DMA queue while W block 0 loads on the sync queue in parallel.
  - W streamed once as 8 column-blocks of 512 cols (32 tiles [128, 512] bf16
    per block); pool holds 2 blocks so the next block prefetches during the
    current one.
  - for nb(8): for k(32): for m(4): matmul(psum[m], x_k[:, m], w[nb,k]).
    m-inner ordering means the first matmul only needs x_0 + w_00 (256 KB of
    DMA) instead of the whole 4 MB xT, and per k-step compute (4 matmuls,
    ~1us) outpaces the 128-256 KB of DMA it unblocks. LDWEIGHTS overlaps
    MATMUL on its own unit, so weight reloads are free.
  - PSUM: 4 m-tiles x 2 bufs x [128, 512] fp32 = all 8 banks; block N+1
    accumulates while block N drains (vector bias-add -> sbuf -> out DMA on
    the scalar queue).
"""

import sys

sys.path.insert(0, "/opt/trn_rl_repo")

import numpy as np
import ml_dtypes

B = 4096
D_IN = 4096
D_OUT = 4096
N_CORES = 8
BS = B // N_CORES  # 512 batch rows per core

K_TILES = D_IN // 128  # 32
N_BLOCK = 512
N_BLOCKS = D_OUT // N_BLOCK  # 8
M_TILES = BS // 128  # 4

_CACHE = {}


def _get_nc():
    if "nc" in _CACHE:
        return _CACHE["nc"]

    import concourse.mybir as mybir
    import concourse.tile as tile
    from concourse import bacc

    nc = bacc.Bacc(None, target_bir_lowering=False)
    xT = nc.declare_dram_parameter(
        "xT", [K_TILES, 128, BS], mybir.dt.bfloat16, isOutput=False
    )
    w = nc.declare_dram_parameter(
        "w", [N_BLOCKS, K_TILES, 128, N_BLOCK], mybir.dt.bfloat16, isOutput=False
    )
    biasr = nc.declare_dram_parameter(
        "biasr", [128, D_OUT], mybir.dt.float32, isOutput=False
    )
    out = nc.declare_dram_parameter("out", [BS, D_OUT], mybir.dt.float32, isOutput=True)

    # ~1.1 column-blocks resident: enough to keep prefetch 4+ k-steps ahead,
    # but throttles block-1 prefetch so it doesn't steal DMA bandwidth from
    # the block-0 startup chase (x + w0 + w1 all in flight exceeds HBM bw).
    W_BUFS = K_TILES + 4
    N_WARM = 14  # PE warm-up matmuls during the initial DMA wait

    with tile.TileContext(nc) as tc:
        with (
            tc.tile_pool(name="xpool", bufs=1) as xpool,
            tc.tile_pool(name="wpool", bufs=W_BUFS) as wpool,
            tc.tile_pool(name="bpool", bufs=1) as bpool,
            tc.tile_pool(name="opool", bufs=4) as opool,
            tc.tile_pool(name="psum", bufs=2, space="PSUM") as psum_pool,
        ):
            # Warm-up: matmuls on zeroed scratch while the first input DMAs are
            # in flight. Keeps the PE busy from t0 so the DVFS p-state is at
            # full clock when real data lands; results are discarded
            # (start=True singleton groups, overwritten by block 0).
            warm_sc = bpool.tile([128, N_BLOCK], mybir.dt.bfloat16, name="warm_sc")
            nc.any.memset(warm_sc[:], 0)
            warm_ps = psum_pool.tile(
                [128, N_BLOCK], mybir.dt.float32, name="warm_ps", tag="ps_0"
            )
            for i in range(N_WARM):
                nc.tensor.matmul(
                    warm_ps[:],
                    warm_sc[:, :128],
                    warm_sc[:],
                    start=True,
                    stop=True,
                )

            # xT tiles resident for the whole kernel (gpsimd queue).
            x_tiles = []
            for k in range(K_TILES):
                xt = xpool.tile(
                    [128, BS], mybir.dt.bfloat16, name=f"x_{k}", tag=f"x_{k}"
                )
                nc.gpsimd.dma_start(xt[:], xT[k])
                x_tiles.append(xt)

            # bias after xT on the same queue (needed only at first drain).
            bias_sb = bpool.tile([128, D_OUT], mybir.dt.float32, name="bias_sb")
            nc.gpsimd.dma_start(bias_sb[:], biasr[:])

            def load_w_tile(nb, k):
                wt = wpool.tile(
                    [128, N_BLOCK], mybir.dt.bfloat16, name=f"w_{nb}_{k}", tag="wt"
                )
                nc.sync.dma_start(wt[:], w[nb, k])
                return wt

            def drain(nb, m, pt):
                ot = opool.tile(
                    [128, N_BLOCK], mybir.dt.float32, name=f"o_{nb}_{m}", tag="o"
                )
                nc.vector.tensor_add(
                    out=ot[:],
                    in0=pt[:],
                    in1=bias_sb[:, nb * N_BLOCK : (nb + 1) * N_BLOCK],
                )
                nc.scalar.dma_start(
                    out[m * 128 : (m + 1) * 128, nb * N_BLOCK : (nb + 1) * N_BLOCK],
                    ot[:],
                )

            for nb in range(N_BLOCKS):
                w_tiles = [load_w_tile(nb, k) for k in range(K_TILES)]
                pts = [
                    psum_pool.tile(
                        [128, N_BLOCK],
                        mybir.dt.float32,
                        name=f"ps_{nb}_{m}",
                        tag=f"ps_{m}",
                    )
                    for m in range(M_TILES)
                ]
                if nb < N_BLOCKS - 1:
                    # m-inner: matches DMA arrival order of w/x tiles (k-major),
                    # so the startup chase never starves the PE.
                    for k in range(K_TILES):
                        for m in range(M_TILES):
                            nc.tensor.matmul(
                                pts[m][:],
                                x_tiles[k][:, m * 128 : (m + 1) * 128],
                                w_tiles[k][:],
                                start=(k == 0),
                                stop=(k == K_TILES - 1),
                            )
                    for m in range(M_TILES):
                        drain(nb, m, pts[m])
                else:
                    # Last block: m-outer so each m-tile finishes early and its
                    # drain overlaps the remaining m-tiles' matmuls (all W tiles
                    # are prefetched by now, so k-sweep bursts are fine).
                    for m in range(M_TILES):
                        for k in range(K_TILES):
                            nc.tensor.matmul(
                                pts[m][:],
                                x_tiles[k][:, m * 128 : (m + 1) * 128],
                                w_tiles[k][:],
                                start=(k == 0),
                                stop=(k == K_TILES - 1),
                            )
                        drain(nb, m, pts[m])
    nc.compile()
    _CACHE["nc"] = nc
    return nc


def _materialize_w(g1, g2, g3):
    W = np.einsum(
        "inr,rjps,skq->ijknpq",
        np.asarray(g1, np.float32),
        np.asarray(g2, np.float32),
        np.asarray(g3, np.float32),
        optimize=True,
    )
    return np.ascontiguousarray(W.reshape(D_IN, D_OUT))


def _make_in_maps(x, g1, g2, g3, bias):
    W = _materialize_w(g1, g2, g3).astype(ml_dtypes.bfloat16)
    # [D_IN, D_OUT] -> [N_BLOCKS, K_TILES, 128, N_BLOCK]
    Wb = np.ascontiguousarray(
        W.reshape(K_TILES, 128, N_BLOCKS, N_BLOCK).transpose(2, 0, 1, 3)
    )
    biasr = np.ascontiguousarray(
        np.broadcast_to(np.asarray(bias, np.float32), (128, D_OUT))
    )
    xb = np.asarray(x, np.float32).astype(ml_dtypes.bfloat16)
    in_maps = []
    for c in range(N_CORES):
        xTc = np.ascontiguousarray(xb[c * BS : (c + 1) * BS, :].T).reshape(
            K_TILES, 128, BS
        )
        in_maps.append({"xT": xTc, "w": Wb, "biasr": biasr})
    return in_maps


def _run(in_maps, trace=False):
    from concourse.bass_utils import run_bass_kernel_spmd

    nc = _get_nc()
    return run_bass_kernel_spmd(nc, in_maps, core_ids=list(range(N_CORES)), trace=trace)


def kernel(x, g1, g2, g3, bias):
    in_maps = _make_in_maps(x, g1, g2, g3, bias)
    res = _run(in_maps)
    out = np.concatenate(
        [res.results[c]["out"] for c in range(N_CORES)], axis=0
    ).astype(np.float32, copy=False)
    return out


# revision 12
# speedup vs baseline: 1.1670x; 1.1670x over previous
"""TT-linear as dense bf16 GEMM, tuned for PE utilization.

Per core: out[512, 4096] = x[512, 4096] @ W[4096, 4096] + bias.
W is materialized on host from the TT cores (TT contraction has the same
FLOP count as the dense GEMM but runs at lower PE efficiency, so dense wins).

Schedule (per core):
  - xT resident in SBUF as 32 tiles [128, 512] bf16 (4 MB), loaded on the
    gpsimd DMA queue while W block 0 loads on the sync queue in parallel.
  - W streamed once as 8 column-blocks of 512 cols (32 tiles [128, 512] bf16
    per block); pool holds 2 blocks so the next block prefetches during the
    current one.
  - for nb(8): for k(32): for m(4): matmul(psum[m], x_k[:, m], w[nb,k]).
    m-inner ordering means the first matmul only needs x_0 + w_00 (256 KB of
    DMA) instead of the whole 4 MB xT, and per k-step compute (4 matmuls,
    ~1us) outpaces the 128-256 KB of DMA it unblocks. LDWEIGHTS overlaps
    MATMUL on its own unit, so weight reloads are free.
  - PSUM: 4 m-tiles x 2 bufs x [128, 512] fp32 = all 8 banks; block N+1
    accumulates while block N drains (vector bias-add -> sbuf -> out DMA on
    the scalar queue).
"""

import sys

sys.path.insert(0, "/opt/trn_rl_repo")

import numpy as np
import ml_dtypes

B = 4096
D_IN = 4096
D_OUT = 4096
N_CORES = 8
BS = B // N_CORES  # 512 batch rows per core

K_TILES = D_IN // 128  # 32
N_BLOCK = 512
N_BLOCKS = D_OUT // N_BLOCK  # 8
M_TILES = BS // 128  # 4

_CACHE = {}


def _get_nc():
    if "nc" in _CACHE:
        return _CACHE["nc"]

    import concourse.mybir as mybir
    import concourse.tile as tile
    from concourse import bacc

    nc = bacc.Bacc(None, target_bir_lowering=False)
    xT = nc.declare_dram_parameter(
        "xT", [K_TILES, 128, BS], mybir.dt.bfloat16, isOutput=False
    )
    w = nc.declare_dram_parameter(
        "w", [N_BLOCKS, K_TILES, 128, N_BLOCK], mybir.dt.bfloat16, isOutput=False
    )
    biasr = nc.declare_dram_parameter(
        "biasr", [128, D_OUT], mybir.dt.float32, isOutput=False
    )
    out = nc.declare_dram_parameter("out", [BS, D_OUT], mybir.dt.float32, isOutput=True)

    # ~1.1 column-blocks resident: enough to keep prefetch 4+ k-steps ahead,
    # but throttles block-1 prefetch so it doesn't steal DMA bandwidth from
    # the block-0 startup chase (x + w0 + w1 all in flight exceeds HBM bw).
    W_BUFS = K_TILES + 4
    N_WARM = 20  # PE warm-up matmuls during the initial DMA wait

    with tile.TileContext(nc) as tc:
        with (
            tc.tile_pool(name="xpool", bufs=1) as xpool,
            tc.tile_pool(name="wpool", bufs=W_BUFS) as wpool,
            tc.tile_pool(name="bpool", bufs=1) as bpool,
            tc.tile_pool(name="opool", bufs=4) as opool,
            tc.tile_pool(name="psum", bufs=2, space="PSUM") as psum_pool,
        ):
            # Warm-up: matmuls on zeroed scratch while the first input DMAs are
            # in flight. Keeps the PE busy from t0 so the DVFS p-state is at
            # full clock when real data lands; results are discarded
            # (start=True singleton groups, overwritten by block 0).
            warm_sc = bpool.tile([128, N_BLOCK], mybir.dt.bfloat16, name="warm_sc")
            nc.any.memset(warm_sc[:], 0)
            warm_ps = psum_pool.tile(
                [128, N_BLOCK], mybir.dt.float32, name="warm_ps", tag="ps_0"
            )
            for i in range(N_WARM):
                nc.tensor.matmul(
                    warm_ps[:],
                    warm_sc[:, :128],
                    warm_sc[:],
                    start=True,
                    stop=True,
                )

            # xT tiles resident for the whole kernel (gpsimd queue).
            x_tiles = []
            for k in range(K_TILES):
                xt = xpool.tile(
                    [128, BS], mybir.dt.bfloat16, name=f"x_{k}", tag=f"x_{k}"
                )
                nc.gpsimd.dma_start(xt[:], xT[k])
                x_tiles.append(xt)

            # bias after xT on the same queue (needed only at first drain).
            bias_sb = bpool.tile([128, D_OUT], mybir.dt.float32, name="bias_sb")
            nc.gpsimd.dma_start(bias_sb[:], biasr[:])

            def load_w_tile(nb, k):
                wt = wpool.tile(
                    [128, N_BLOCK], mybir.dt.bfloat16, name=f"w_{nb}_{k}", tag="wt"
                )
                nc.sync.dma_start(wt[:], w[nb, k])
                return wt

            def drain(nb, m, pt):
                ot = opool.tile(
                    [128, N_BLOCK], mybir.dt.float32, name=f"o_{nb}_{m}", tag="o"
                )
                nc.vector.tensor_add(
                    out=ot[:],
                    in0=pt[:],
                    in1=bias_sb[:, nb * N_BLOCK : (nb + 1) * N_BLOCK],
                )
                nc.scalar.dma_start(
                    out[m * 128 : (m + 1) * 128, nb * N_BLOCK : (nb + 1) * N_BLOCK],
                    ot[:],
                )

            for nb in range(N_BLOCKS):
                w_tiles = [load_w_tile(nb, k) for k in range(K_TILES)]
                pts = [
                    psum_pool.tile(
                        [128, N_BLOCK],
                        mybir.dt.float32,
                        name=f"ps_{nb}_{m}",
                        tag=f"ps_{m}",
                    )
                    for m in range(M_TILES)
                ]
                if nb < N_BLOCKS - 1:
                    # m-inner: matches DMA arrival order of w/x tiles (k-major),
                    # so the startup chase never starves the PE.
                    for k in range(K_TILES):
                        for m in range(M_TILES):
                            nc.tensor.matmul(
                                pts[m][:],
                                x_tiles[k][:, m * 128 : (m + 1) * 128],
                                w_tiles[k][:],
                                start=(k == 0),
                                stop=(k == K_TILES - 1),
                            )
                    for m in range(M_TILES):
                        drain(nb, m, pts[m])
                else:
                    # Last block: m-outer so each m-tile finishes early and its
                    # drain overlaps the remaining m-tiles' matmuls (all W tiles
                    # are prefetched by now, so k-sweep bursts are fine).
                    for m in range(M_TILES):
                        for k in range(K_TILES):
                            nc.tensor.matmul(
                                pts[m][:],
                                x_tiles[k][:, m * 128 : (m + 1) * 128],
                                w_tiles[k][:],
                                start=(k == 0),
                                stop=(k == K_TILES - 1),
                            )
                        drain(nb, m, pts[m])
    nc.compile()
    _CACHE["nc"] = nc
    return nc


def _materialize_w(g1, g2, g3):
    W = np.einsum(
        "inr,rjps,skq->ijknpq",
        np.asarray(g1, np.float32),
        np.asarray(g2, np.float32),
        np.asarray(g3, np.float32),
        optimize=True,
    )
    return np.ascontiguousarray(W.reshape(D_IN, D_OUT))


def _make_in_maps(x, g1, g2, g3, bias):
    W = _materialize_w(g1, g2, g3).astype(ml_dtypes.bfloat16)
    # [D_IN, D_OUT] -> [N_BLOCKS, K_TILES, 128, N_BLOCK]
    Wb = np.ascontiguousarray(
        W.reshape(K_TILES, 128, N_BLOCKS, N_BLOCK).transpose(2, 0, 1, 3)
    )
    biasr = np.ascontiguousarray(
        np.broadcast_to(np.asarray(bias, np.float32), (128, D_OUT))
    )
    xb = np.asarray(x, np.float32).astype(ml_dtypes.bfloat16)
    in_maps = []
    for c in range(N_CORES):
        xTc = np.ascontiguousarray(xb[c * BS : (c + 1) * BS, :].T).reshape(
            K_TILES, 128, BS
        )
        in_maps.append({"xT": xTc, "w": Wb, "biasr": biasr})
    return in_maps


def _run(in_maps, trace=False):
    from concourse.bass_utils import run_bass_kernel_spmd

    nc = _get_nc()
    return run_bass_kernel_spmd(nc, in_maps, core_ids=list(range(N_CORES)), trace=trace)


def kernel(x, g1, g2, g3, bias):
    in_maps = _make_in_maps(x, g1, g2, g3, bias)
    res = _run(in_maps)
    out = np.concatenate(
        [res.results[c]["out"] for c in range(N_CORES)], axis=0
    ).astype(np.float32, copy=False)
    return out
